# revision 14
# baseline (speedup 1.0000x reference)
"""MoE (top-2 of 8 experts, GELU MLP) on 8 Trainium2 NeuronCores.

Strategy (expert-parallel, per sharding hint):
  Launch 1 (gate, data-parallel): each core takes a 1024-token shard and
    computes per-token combine weights comb[t, e] (softmax over the top-2
    expert logits, scattered to the selected experts) fully on device.
    Gate logits stay fp32: low-precision logits flip near-tie top-2
    selections, which is catastrophically expensive in output error.
  Host glue: build per-expert token index lists from comb (pure
    gather/scatter data movement), gather x columns per expert, pre-cast
    the expert-side tensors to bf16 (halves HBM traffic; matmuls run in
    bf16 with fp32 PSUM accumulation anyway).
  Launch 2 (experts): core e runs its expert's GELU MLP over the tokens
    routed to it (padded to a common capacity C) entirely in bf16 SBUF,
    scales by the combine weight, returns y^T per expert as bf16.
  Host glue: upcast + scatter-add the (disjoint-per-expert) rows into the
    fp32 output.
"""

import sys

import ml_dtypes
import numpy as np

try:
    import concourse.bass as bass  # noqa: F401
except ImportError:  # container default location of the concourse repo
    sys.path.insert(0, "/opt/trn_rl_repo")

import concourse.bass as bass
import concourse.tile as tile
from concourse import bacc, mybir
from concourse.bass_utils import run_bass_kernel_spmd

F32 = mybir.dt.float32
BF16 = mybir.dt.bfloat16
AF = mybir.ActivationFunctionType
ALU = mybir.AluOpType
NPBF16 = ml_dtypes.bfloat16

E = 8          # experts
D = 1024       # d_model
F = 2048       # d_ff
T = 8192       # tokens (4*2048)
NCORES = 8
TSHARD = T // NCORES
P = 128


def _new_nc():
    return bacc.Bacc("TRN2", target_bir_lowering=False, debug=False,
                     num_devices=NCORES)


def build_gate_nc(repeat: int = 1):
    """Per core: xt [D, TSHARD] f32 (x^T token shard), wg [D, E] f32
    -> lgt [E, TSHARD] f32 gate logits (no bias; host adds bg).

    The matmul runs in float32r (FP22): 4x the fp32 throughput with enough
    mantissa (13 bits) that top-2 selection never flips vs the fp32
    reference (bf16 would flip ~0.3% of tokens = ~3% output error).
    Top-2 + softmax + scatter into comb happen on the host: they are
    routing logic on a [T, 8] table, and doing them here serializes ~14
    tiny vector ops per rep.
    """
    nc = _new_nc()
    F32R = mybir.dt.float32r
    xt = nc.dram_tensor("xt", [D, TSHARD], F32R, kind="ExternalInput").ap()
    wg = nc.dram_tensor("wg", [D, E], F32R, kind="ExternalInput").ap()
    lgt = nc.dram_tensor("lgt", [E, TSHARD], F32, kind="ExternalOutput").ap()

    KD = D // P          # 8 contraction tiles
    NB = TSHARD // 512   # 2 psum-sized token blocks

    with tile.TileContext(nc) as tc:
        with (
            tc.tile_pool(name="io", bufs=2) as io,
            tc.tile_pool(name="ob", bufs=2) as ob,
            tc.tile_pool(name="psum", bufs=2, space="PSUM") as psum,
        ):
            for _ in range(repeat):
                wgsb = io.tile([P, KD, E], F32R, tag="wgsb")
                nc.sync.dma_start(
                    wgsb[:], wg.rearrange("(ko ki) e -> ki ko e", ki=P))
                xt3 = xt.rearrange("(ko ki) n -> ki ko n", ki=P)
                xsbs = []
                for k in range(KD):
                    xsb = io.tile([P, TSHARD], F32R, tag=f"xsb_{k}",
                                  name=f"xsb_{k}")
                    xsbs.append(xsb)
                for b in range(NB):  # block-major so block 0 lands first
                    for k in range(KD):
                        nc.sync.dma_start(
                            xsbs[k][:, b * 512:(b + 1) * 512],
                            xt3[:, k, b * 512:(b + 1) * 512])
                for b in range(NB):
                    ps = psum.tile([E, 512], F32, tag="ps")
                    for k in range(KD):
                        nc.tensor.matmul(
                            ps[:], wgsb[:, k, :],
                            xsbs[k][:, b * 512:(b + 1) * 512],
                            start=(k == 0), stop=(k == KD - 1))
                    lg = ob.tile([E, 512], F32, tag="lg")
                    nc.scalar.activation(lg[:], ps[:], AF.Copy)
                    nc.sync.dma_start(lgt[:, b * 512:(b + 1) * 512], lg[:])
    nc.compile()
    return nc


def gate_comb(results, bg):
    """Host: logits -> top-2 -> softmax -> comb [T, E] (reference order)."""
    lg = np.concatenate([r["lgt"].T for r in results], axis=0)  # [T, E]
    lg = lg + bg[None, :]
    sel = np.argsort(-lg, axis=1, kind="stable")[:, :2]
    top = np.take_along_axis(lg, sel, 1).astype(np.float32)
    w = np.exp(top - top.max(1, keepdims=True))
    w /= w.sum(1, keepdims=True)
    comb = np.zeros((T, E), np.float32)
    rows = np.arange(T)
    for k in range(2):
        comb[rows, sel[:, k]] += w[:, k]
    return comb


def build_expert_nc(C: int, repeat: int = 1, ntile: int = 512,
                    parts: str = "full"):
    """Per core: one expert's GELU MLP over C (padded) routed tokens.

    All big tensors arrive pre-cast to bf16 from the host. xgt [D, C] bf16
    gathered x^T; wb [P, C] f32 combine weight broadcast across partitions;
    w1 [D, F] bf16; b1c [P, F//P] f32; w2 [F, D] bf16; b2c [P, D//P] f32
    -> yt [D, C] bf16 where yt[:, j] = wb[j] * (gelu(x_j @ W1 + b1) @ W2 + b2).
    """
    assert C % 8 == 0
    nc = _new_nc()
    xgt = nc.dram_tensor("xgt", [D, C], BF16, kind="ExternalInput").ap()
    wb = nc.dram_tensor("wb", [P, C], F32, kind="ExternalInput").ap()
    w1 = nc.dram_tensor("w1", [D, F], BF16, kind="ExternalInput").ap()
    b1c = nc.dram_tensor("b1c", [P, F // P], F32, kind="ExternalInput").ap()
    w2 = nc.dram_tensor("w2", [F, D], BF16, kind="ExternalInput").ap()
    b2c = nc.dram_tensor("b2c", [P, D // P], F32, kind="ExternalInput").ap()
    yt = nc.dram_tensor("yt", [D, C], BF16, kind="ExternalOutput").ap()

    KD = D // P    # 8  k-tiles for x @ W1
    KF = F // P    # 16 k-tiles for h @ W2
    MF = F // P    # 16 dff output tiles
    MD = D // P    # 8  dmodel output tiles
    NTILE = ntile
    ntok = [(n0, min(NTILE, C - n0)) for n0 in range(0, C, NTILE)]
    scale = NTILE // 512  # keep SBUF/PSUM footprint constant across ntile

    xgt_r = xgt.rearrange("(ko ki) n -> ki ko n", ki=P)
    with tile.TileContext(nc) as tc:
        with (
            tc.tile_pool(name="res", bufs=1) as res,
            tc.tile_pool(name="dbl", bufs=2) as dbl,
            tc.tile_pool(name="xio", bufs=3) as xio,
            tc.tile_pool(name="hbuf", bufs=max(1, 2 // scale)) as hbuf,
            tc.tile_pool(name="obuf", bufs=max(2, 4 // scale)) as obuf,
            tc.tile_pool(name="psum", bufs=8 // scale, space="PSUM") as psum,
        ):
            do_io = parts in ("full", "io")
            do_mm = parts in ("full", "compute")
            for _ in range(repeat):
                b1sb = dbl.tile([P, F // P], F32, tag="b1sb")
                nc.sync.dma_start(b1sb[:], b1c[:])
                b2sb = dbl.tile([P, D // P], F32, tag="b2sb")
                nc.sync.dma_start(b2sb[:], b2c[:])

                # W1/W2 stay resident for the whole rep; x streams in
                # per-n-tile chunks (xio rotates 3 bufs) so the next rep's
                # DMAs aren't blocked on whole-rep tile liveness. W2/wb are
                # double-buffered (dbl) because they stay live to the very
                # end of the rep. DMA issue order = first-use order.
                def xc_fetch(it):
                    n0, nn = ntok[it]
                    xc = xio.tile([P, KD, NTILE], BF16, tag="xc", name="xc")
                    if do_io:
                        if it == 0:
                            # per-k slices so the first matmul starts after
                            # ~0.13MB instead of the full 1MB chunk
                            for k in range(KD):
                                nc.sync.dma_start(
                                    xc[:, k, :nn],
                                    xgt_r[:, k, n0:n0 + nn])
                        else:
                            nc.sync.dma_start(xc[:, :, :nn],
                                              xgt_r[:, :, n0:n0 + nn])
                    return xc

                # prefetch depth 3 = xio bufs; later chunks are fetched
                # in-loop (after their buffer's WAR clears) so a blocked
                # DMA never sits ahead of the w2/wb transfers in a queue.
                # Issue order = first-use order: x tile 0, W1, then the
                # tile 1-2 prefetches, W2, wb.
                xcs = [xc_fetch(0)]
                w1sb = [res.tile([P, F], BF16, tag=f"w1_{k}", name=f"w1_{k}")
                        for k in range(KD)]
                w2sb = [dbl.tile([P, D], BF16, tag=f"w2_{k}", name=f"w2_{k}")
                        for k in range(KF)]
                wbsb = dbl.tile([P, C], F32, tag="wbsb")
                if do_io:
                    # column halves: the first psum group (mf=0) only needs
                    # w1[:, :128], so make half the columns land first
                    for k in range(KD):
                        nc.sync.dma_start(w1sb[k][:, :F // 2],
                                          w1[k * P:(k + 1) * P, :F // 2])
                    for k in range(KD):
                        nc.sync.dma_start(w1sb[k][:, F // 2:],
                                          w1[k * P:(k + 1) * P, F // 2:])
                xcs += [xc_fetch(it) for it in range(1, min(3, len(ntok)))]
                if do_io:
                    for k in range(KF):
                        nc.sync.dma_start(w2sb[k][:],
                                          w2[k * P:(k + 1) * P, :])
                    nc.sync.dma_start(wbsb[:], wb[:])

                for it, (n0, nn) in enumerate(ntok if do_mm else []):
                    xc = xcs[it]
                    hs = []
                    for mf in range(MF):
                        ps = psum.tile([P, NTILE], F32, tag="ps")
                        for k in range(KD):
                            nc.tensor.matmul(
                                ps[:, :nn],
                                w1sb[k][:, mf * P:(mf + 1) * P],
                                xc[:, k, :nn],
                                start=(k == 0), stop=(k == KD - 1))
                        h = hbuf.tile([P, NTILE], BF16, tag=f"h_{mf}")
                        nc.scalar.activation(h[:, :nn], ps[:, :nn],
                                             AF.Gelu_apprx_tanh,
                                             bias=b1sb[:, mf:mf + 1])
                        hs.append(h)
                    if it + 3 < len(ntok):
                        xcs.append(xc_fetch(it + 3))
                    for md in range(MD):
                        ps2 = psum.tile([P, NTILE], F32, tag="ps")
                        for k in range(KF):
                            nc.tensor.matmul(
                                ps2[:, :nn],
                                w2sb[k][:, md * P:(md + 1) * P],
                                hs[k][:, :nn],
                                start=(k == 0), stop=(k == KF - 1))
                        # yw = (y + b2) * w  in one DVE op
                        yw = obuf.tile([P, NTILE], BF16, tag="yw")
                        nc.vector.scalar_tensor_tensor(
                            yw[:, :nn], ps2[:, :nn], b2sb[:, md:md + 1],
                            wbsb[:, n0:n0 + nn], op0=ALU.add, op1=ALU.mult)
                        nc.sync.dma_start(yt[md * P:(md + 1) * P, n0:n0 + nn],
                                          yw[:, :nn])
    nc.compile()
    return nc


def _run(nc, in_maps):
    res = run_bass_kernel_spmd(nc, in_maps, core_ids=list(range(NCORES)))
    return res.results


def gate_in_maps(xT, Wg, bg):
    wg = np.ascontiguousarray(Wg, dtype=np.float32)
    return [
        {"xt": np.ascontiguousarray(xT[:, c * TSHARD:(c + 1) * TSHARD]),
         "wg": wg}
        for c in range(NCORES)
    ]


def routing_from_comb(comb):
    idxs = [np.nonzero(comb[:, e])[0] for e in range(E)]
    maxn = max(len(i) for i in idxs)
    # capacity only needs 8-element alignment (matmul free dim is arbitrary;
    # 16B DMA rows); 128-alignment would waste ~5% expert compute
    C = max(((maxn + 7) // 8) * 8, P)
    return idxs, C


def expert_in_maps(xT, comb, idxs, C, W1, b1, W2, b2):
    xTb = xT.astype(NPBF16)
    in_maps = []
    for e in range(E):
        idx = idxs[e]
        n = len(idx)
        xgt = np.zeros((D, C), NPBF16)
        xgt[:, :n] = xTb[:, idx]
        wbe = np.zeros((P, C), np.float32)
        wbe[:, :n] = comb[idx, e][None, :]
        in_maps.append({
            "xgt": xgt,
            "wb": wbe,
            "w1": np.ascontiguousarray(W1[e].astype(NPBF16)),
            "b1c": np.ascontiguousarray(
                b1[e].reshape(F // P, P).T, dtype=np.float32),
            "w2": np.ascontiguousarray(W2[e].astype(NPBF16)),
            "b2c": np.ascontiguousarray(
                b2[e].reshape(D // P, P).T, dtype=np.float32),
        })
    return in_maps


def combine_outputs(outs, idxs, x_shape):
    out = np.zeros((T, D), np.float32)
    for e in range(E):
        idx = idxs[e]
        out[idx] += outs[e]["yt"][:, :len(idx)].T.astype(np.float32)
    return out.reshape(x_shape)


def kernel(x, Wg, bg, W1, b1, W2, b2):
    x = np.asarray(x, dtype=np.float32)
    Wg = np.asarray(Wg, dtype=np.float32)
    bg = np.asarray(bg, dtype=np.float32)
    W1 = np.asarray(W1, dtype=np.float32)
    b1 = np.asarray(b1, dtype=np.float32)
    W2 = np.asarray(W2, dtype=np.float32)
    b2 = np.asarray(b2, dtype=np.float32)

    xf = x.reshape(T, D)
    xT = np.ascontiguousarray(xf.T)

    nc_g = build_gate_nc()
    comb = gate_comb(_run(nc_g, gate_in_maps(xT, Wg, bg)), bg)

    idxs, C = routing_from_comb(comb)
    nc_e = build_expert_nc(C)
    outs = _run(nc_e, expert_in_maps(xT, comb, idxs, C, W1, b1, W2, b2))
    return combine_outputs(outs, idxs, x.shape)


# revision 16
# speedup vs baseline: 1.2808x; 1.2808x over previous
"""MoE (top-2 of 8 experts, GELU MLP) on 8 Trainium2 NeuronCores.

Strategy (expert-parallel, per sharding hint):
  Launch 1 (gate, data-parallel): each core takes a 1024-token shard and
    computes per-token combine weights comb[t, e] (softmax over the top-2
    expert logits, scattered to the selected experts) fully on device.
    Gate logits stay fp32: low-precision logits flip near-tie top-2
    selections, which is catastrophically expensive in output error.
  Host glue: build per-expert token index lists from comb (pure
    gather/scatter data movement), gather x columns per expert, pre-cast
    the expert-side tensors to bf16 (halves HBM traffic; matmuls run in
    bf16 with fp32 PSUM accumulation anyway).
  Launch 2 (experts): core e runs its expert's GELU MLP over the tokens
    routed to it (padded to a common capacity C) entirely in bf16 SBUF,
    scales by the combine weight, returns y^T per expert as bf16.
  Host glue: upcast + scatter-add the (disjoint-per-expert) rows into the
    fp32 output.
"""

import sys

import ml_dtypes
import numpy as np

try:
    import concourse.bass as bass  # noqa: F401
except ImportError:  # container default location of the concourse repo
    sys.path.insert(0, "/opt/trn_rl_repo")

import concourse.bass as bass
import concourse.tile as tile
from concourse import bacc, mybir
from concourse.bass_utils import run_bass_kernel_spmd

F32 = mybir.dt.float32
BF16 = mybir.dt.bfloat16
AF = mybir.ActivationFunctionType
ALU = mybir.AluOpType
NPBF16 = ml_dtypes.bfloat16

E = 8          # experts
D = 1024       # d_model
F = 2048       # d_ff
T = 8192       # tokens (4*2048)
NCORES = 8
TSHARD = T // NCORES
P = 128


def _new_nc():
    return bacc.Bacc("TRN2", target_bir_lowering=False, debug=False,
                     num_devices=NCORES)


def build_gate_nc(repeat: int = 1):
    """Per core: xt [D, TSHARD] f32 (x^T token shard), wg [D, E] f32
    -> lgt [E, TSHARD] f32 gate logits (no bias; host adds bg).

    The matmul runs in float32r (FP22): 4x the fp32 throughput with enough
    mantissa (13 bits) that top-2 selection never flips vs the fp32
    reference (bf16 would flip ~0.3% of tokens = ~3% output error).
    Top-2 + softmax + scatter into comb happen on the host: they are
    routing logic on a [T, 8] table, and doing them here serializes ~14
    tiny vector ops per rep.
    """
    nc = _new_nc()
    F32R = mybir.dt.float32r
    xt = nc.dram_tensor("xt", [D, TSHARD], F32R, kind="ExternalInput").ap()
    wg = nc.dram_tensor("wg", [D, E], F32R, kind="ExternalInput").ap()
    lgt = nc.dram_tensor("lgt", [E, TSHARD], F32, kind="ExternalOutput").ap()

    KD = D // P          # 8 contraction tiles
    NB = TSHARD // 512   # 2 psum-sized token blocks

    with tile.TileContext(nc) as tc:
        with (
            tc.tile_pool(name="io", bufs=2) as io,
            tc.tile_pool(name="ob", bufs=2) as ob,
            tc.tile_pool(name="psum", bufs=2, space="PSUM") as psum,
        ):
            for _ in range(repeat):
                wgsb = io.tile([P, KD, E], F32R, tag="wgsb")
                nc.sync.dma_start(
                    wgsb[:], wg.rearrange("(ko ki) e -> ki ko e", ki=P))
                xt3 = xt.rearrange("(ko ki) n -> ki ko n", ki=P)
                xsbs = []
                for k in range(KD):
                    xsb = io.tile([P, TSHARD], F32R, tag=f"xsb_{k}",
                                  name=f"xsb_{k}")
                    xsbs.append(xsb)
                for b in range(NB):  # block-major so block 0 lands first
                    for k in range(KD):
                        nc.sync.dma_start(
                            xsbs[k][:, b * 512:(b + 1) * 512],
                            xt3[:, k, b * 512:(b + 1) * 512])
                for b in range(NB):
                    ps = psum.tile([E, 512], F32, tag="ps")
                    for k in range(KD):
                        nc.tensor.matmul(
                            ps[:], wgsb[:, k, :],
                            xsbs[k][:, b * 512:(b + 1) * 512],
                            start=(k == 0), stop=(k == KD - 1))
                    lg = ob.tile([E, 512], F32, tag="lg")
                    nc.scalar.activation(lg[:], ps[:], AF.Copy)
                    nc.sync.dma_start(lgt[:, b * 512:(b + 1) * 512], lg[:])
    nc.compile()
    return nc


def gate_comb(results, bg):
    """Host: logits -> top-2 -> softmax -> comb [T, E] (reference order)."""
    lg = np.concatenate([r["lgt"].T for r in results], axis=0)  # [T, E]
    lg = lg + bg[None, :]
    sel = np.argsort(-lg, axis=1, kind="stable")[:, :2]
    top = np.take_along_axis(lg, sel, 1).astype(np.float32)
    w = np.exp(top - top.max(1, keepdims=True))
    w /= w.sum(1, keepdims=True)
    comb = np.zeros((T, E), np.float32)
    rows = np.arange(T)
    for k in range(2):
        comb[rows, sel[:, k]] += w[:, k]
    return comb


def build_expert_nc(C: int, repeat: int = 1, ntile: int = 512,
                    parts: str = "full"):
    """Per core: one expert's GELU MLP over C (padded) routed tokens.

    All big tensors arrive pre-cast to bf16 from the host. xgt [D, C] bf16
    gathered x^T; wb [P, C] f32 combine weight broadcast across partitions;
    w1 [D, F] bf16; b1c [P, F//P] f32; w2 [F, D] bf16; b2c [P, D//P] f32
    -> yt [D, C] bf16 where yt[:, j] = wb[j] * (gelu(x_j @ W1 + b1) @ W2 + b2).
    """
    assert C % 8 == 0
    nc = _new_nc()
    xgt = nc.dram_tensor("xgt", [D, C], BF16, kind="ExternalInput").ap()
    wb = nc.dram_tensor("wb", [P, C], F32, kind="ExternalInput").ap()
    w1 = nc.dram_tensor("w1", [D, F], BF16, kind="ExternalInput").ap()
    b1c = nc.dram_tensor("b1c", [P, F // P], F32, kind="ExternalInput").ap()
    w2 = nc.dram_tensor("w2", [F, D], BF16, kind="ExternalInput").ap()
    b2c = nc.dram_tensor("b2c", [P, D // P], F32, kind="ExternalInput").ap()
    yt = nc.dram_tensor("yt", [D, C], BF16, kind="ExternalOutput").ap()

    KD = D // P    # 8  k-tiles for x @ W1
    KF = F // P    # 16 k-tiles for h @ W2
    MF = F // P    # 16 dff output tiles
    MD = D // P    # 8  dmodel output tiles
    NTILE = ntile
    ntok = [(n0, min(NTILE, C - n0)) for n0 in range(0, C, NTILE)]
    scale = NTILE // 512  # keep SBUF/PSUM footprint constant across ntile

    xgt_r = xgt.rearrange("(ko ki) n -> ki ko n", ki=P)
    with tile.TileContext(nc) as tc:
        with (
            tc.tile_pool(name="res", bufs=1) as res,
            tc.tile_pool(name="dbl", bufs=2) as dbl,
            tc.tile_pool(name="xio", bufs=3) as xio,
            tc.tile_pool(name="hbuf", bufs=max(1, 2 // scale)) as hbuf,
            tc.tile_pool(name="obuf", bufs=max(2, 4 // scale)) as obuf,
            tc.tile_pool(name="psum", bufs=8 // scale, space="PSUM") as psum,
        ):
            do_io = parts in ("full", "io")
            do_mm = parts in ("full", "compute")
            for _ in range(repeat):
                b1sb = dbl.tile([P, F // P], F32, tag="b1sb")
                nc.sync.dma_start(b1sb[:], b1c[:])
                b2sb = dbl.tile([P, D // P], F32, tag="b2sb")
                nc.sync.dma_start(b2sb[:], b2c[:])

                # W1/W2 stay resident for the whole rep; x streams in
                # per-n-tile chunks (xio rotates 3 bufs) so the next rep's
                # DMAs aren't blocked on whole-rep tile liveness. W2/wb are
                # double-buffered (dbl) because they stay live to the very
                # end of the rep. DMA issue order = first-use order.
                def xc_fetch(it):
                    n0, nn = ntok[it]
                    xc = xio.tile([P, KD, NTILE], BF16, tag="xc", name="xc")
                    if do_io:
                        if it == 0:
                            # per-k slices so the first matmul starts after
                            # ~0.13MB instead of the full 1MB chunk
                            for k in range(KD):
                                nc.sync.dma_start(
                                    xc[:, k, :nn],
                                    xgt_r[:, k, n0:n0 + nn])
                        else:
                            nc.sync.dma_start(xc[:, :, :nn],
                                              xgt_r[:, :, n0:n0 + nn])
                    return xc

                # prefetch depth 3 = xio bufs; later chunks are fetched
                # in-loop (after their buffer's WAR clears) so a blocked
                # DMA never sits ahead of the w2/wb transfers in a queue.
                # Issue order = first-use order: x tile 0, W1, then the
                # tile 1-2 prefetches, W2, wb.
                xcs = [xc_fetch(0)]
                w1sb = [res.tile([P, F], BF16, tag=f"w1_{k}", name=f"w1_{k}")
                        for k in range(KD)]
                w2sb = [dbl.tile([P, D], BF16, tag=f"w2_{k}", name=f"w2_{k}")
                        for k in range(KF)]
                wbsb = dbl.tile([P, C], F32, tag="wbsb")
                if do_io:
                    # column quarters: the first psum group (mf=0) only
                    # needs w1[:, :128], so make early columns land first
                    for q in range(4):
                        for k in range(KD):
                            nc.sync.dma_start(
                                w1sb[k][:, q * F // 4:(q + 1) * F // 4],
                                w1[k * P:(k + 1) * P,
                                   q * F // 4:(q + 1) * F // 4])
                xcs += [xc_fetch(it) for it in range(1, min(3, len(ntok)))]
                if do_io:
                    for k in range(KF):
                        nc.sync.dma_start(w2sb[k][:],
                                          w2[k * P:(k + 1) * P, :])
                    nc.sync.dma_start(wbsb[:], wb[:])

                def phase1(it):
                    n0, nn = ntok[it]
                    xc = xcs[it]
                    hs = []
                    for mf in range(MF):
                        ps = psum.tile([P, NTILE], F32, tag="ps")
                        for k in range(KD):
                            nc.tensor.matmul(
                                ps[:, :nn],
                                w1sb[k][:, mf * P:(mf + 1) * P],
                                xc[:, k, :nn],
                                start=(k == 0), stop=(k == KD - 1))
                        h = hbuf.tile([P, NTILE], BF16, tag=f"h_{mf}",
                                      name=f"h_{mf}")
                        nc.scalar.activation(h[:, :nn], ps[:, :nn],
                                             AF.Gelu_apprx_tanh,
                                             bias=b1sb[:, mf:mf + 1])
                        hs.append(h)
                    if it + 3 < len(ntok):
                        xcs.append(xc_fetch(it + 3))
                    return hs

                def phase2(it, hs):
                    n0, nn = ntok[it]
                    for md in range(MD):
                        ps2 = psum.tile([P, NTILE], F32, tag="ps")
                        for k in range(KF):
                            nc.tensor.matmul(
                                ps2[:, :nn],
                                w2sb[k][:, md * P:(md + 1) * P],
                                hs[k][:, :nn],
                                start=(k == 0), stop=(k == KF - 1))
                        # yw = (y + b2) * w  in one DVE op
                        yw = obuf.tile([P, NTILE], BF16, tag="yw")
                        nc.vector.scalar_tensor_tensor(
                            yw[:, :nn], ps2[:, :nn], b2sb[:, md:md + 1],
                            wbsb[:, n0:n0 + nn], op0=ALU.add, op1=ALU.mult)
                        nc.sync.dma_start(yt[md * P:(md + 1) * P, n0:n0 + nn],
                                          yw[:, :nn])

                # phase2 runs one tile behind phase1: tile 0's phase2 never
                # sits in the (in-order) tensor stream before W2 arrives
                prev = None
                for it in range(len(ntok) if do_mm else 0):
                    hs = phase1(it)
                    if prev is not None:
                        phase2(prev[0], prev[1])
                    prev = (it, hs)
                if prev is not None:
                    phase2(prev[0], prev[1])
    nc.compile()
    return nc


def _run(nc, in_maps):
    res = run_bass_kernel_spmd(nc, in_maps, core_ids=list(range(NCORES)))
    return res.results


def gate_in_maps(xT, Wg, bg):
    wg = np.ascontiguousarray(Wg, dtype=np.float32)
    return [
        {"xt": np.ascontiguousarray(xT[:, c * TSHARD:(c + 1) * TSHARD]),
         "wg": wg}
        for c in range(NCORES)
    ]


def routing_from_comb(comb):
    idxs = [np.nonzero(comb[:, e])[0] for e in range(E)]
    maxn = max(len(i) for i in idxs)
    # capacity only needs 8-element alignment (matmul free dim is arbitrary;
    # 16B DMA rows); 128-alignment would waste ~5% expert compute
    C = max(((maxn + 7) // 8) * 8, P)
    return idxs, C


def expert_in_maps(xT, comb, idxs, C, W1, b1, W2, b2):
    xTb = xT.astype(NPBF16)
    in_maps = []
    for e in range(E):
        idx = idxs[e]
        n = len(idx)
        xgt = np.zeros((D, C), NPBF16)
        xgt[:, :n] = xTb[:, idx]
        wbe = np.zeros((P, C), np.float32)
        wbe[:, :n] = comb[idx, e][None, :]
        in_maps.append({
            "xgt": xgt,
            "wb": wbe,
            "w1": np.ascontiguousarray(W1[e].astype(NPBF16)),
            "b1c": np.ascontiguousarray(
                b1[e].reshape(F // P, P).T, dtype=np.float32),
            "w2": np.ascontiguousarray(W2[e].astype(NPBF16)),
            "b2c": np.ascontiguousarray(
                b2[e].reshape(D // P, P).T, dtype=np.float32),
        })
    return in_maps


def combine_outputs(outs, idxs, x_shape):
    out = np.zeros((T, D), np.float32)
    for e in range(E):
        idx = idxs[e]
        out[idx] += outs[e]["yt"][:, :len(idx)].T.astype(np.float32)
    return out.reshape(x_shape)


def kernel(x, Wg, bg, W1, b1, W2, b2):
    x = np.asarray(x, dtype=np.float32)
    Wg = np.asarray(Wg, dtype=np.float32)
    bg = np.asarray(bg, dtype=np.float32)
    W1 = np.asarray(W1, dtype=np.float32)
    b1 = np.asarray(b1, dtype=np.float32)
    W2 = np.asarray(W2, dtype=np.float32)
    b2 = np.asarray(b2, dtype=np.float32)

    xf = x.reshape(T, D)
    xT = np.ascontiguousarray(xf.T)

    nc_g = build_gate_nc()
    comb = gate_comb(_run(nc_g, gate_in_maps(xT, Wg, bg)), bg)

    idxs, C = routing_from_comb(comb)
    nc_e = build_expert_nc(C)
    outs = _run(nc_e, expert_in_maps(xT, comb, idxs, C, W1, b1, W2, b2))
    return combine_outputs(outs, idxs, x.shape)
